# revision 1
# baseline (speedup 1.0000x reference)
# Trainium2 Bass kernel for nn_MultiHeadGridAttention1d (multi-head grid attention).
# 8 cores = (batch 0..4) x (head-half): per-core 4 heads attention + partial proj;
# host sums the two partials per batch.
import os, sys
import numpy as np
import ml_dtypes

if '/opt/trn_rl_repo' not in sys.path:
    sys.path.insert(0, '/opt/trn_rl_repo')

import concourse.bass as bass
import concourse.tile as tile
from concourse import bacc, mybir
from concourse import bass_utils

NH, KD, HD, C = 8, 32, 64, 512
W0 = 12; W4 = W0**4; G3 = W0**3; T = G3//4
SCALE = KD ** -0.5
PT = 432; NPT = W4 // PT
bf16 = mybir.dt.bfloat16; f32 = mybir.dt.float32

def mk(ap, dims, off=0):
    return bass.AP(tensor=ap.tensor, offset=ap.offset + off, ap=dims)

def build_program():
    nc = bacc.Bacc("TRN2", target_bir_lowering=False, debug=False, num_devices=8)
    def din(name, shape, dt=bf16):
        return nc.dram_tensor(name, shape, dt, kind="ExternalInput").ap()
    xb    = din("xb", [4, 128, W4])
    wconv = din("wconv", [4, 128, 576])
    bconv = din("bconv", [640], f32)
    def scr(name, n, dt=bf16):
        return nc.dram_tensor(name, [int(n)], dt, kind="Internal").ap()
    q1d = nc.dram_tensor("q1d", [128*W4], bf16, kind="ExternalOutput").ap()
    q2d = nc.dram_tensor("q2d", [128*W4], bf16, kind="ExternalOutput").ap()
    vd  = nc.dram_tensor("vd", [256*W4], bf16, kind="ExternalOutput").ap()
    ksd = nc.dram_tensor("ksd", [64*W4], bf16, kind="ExternalOutput").ap()

    EXPT = mybir.ActivationFunctionType.Exp
    COPYT = mybir.ActivationFunctionType.Copy
    IDENT = mybir.ActivationFunctionType.Identity
    AL = mybir.AluOpType
    import contextlib
    ctx = contextlib.ExitStack()
    with tile.TileContext(nc) as tc, ctx:
        const = ctx.enter_context(tc.tile_pool(name="const", bufs=1))
        sb  = ctx.enter_context(tc.tile_pool(name="sb", bufs=3))
        big = ctx.enter_context(tc.tile_pool(name="big", bufs=1))
        ps  = ctx.enter_context(tc.tile_pool(name="ps", bufs=2, space="PSUM"))
        ps2 = ctx.enter_context(tc.tile_pool(name="ps2", bufs=4, space="PSUM"))

        # ---------------- conv ----------------
        bcol = const.tile([128, 5], f32)
        for mch in range(5):
            nc.sync.dma_start(bcol[:, mch:mch+1], mk(bconv, [[1, 128], [1, 1]], mch*128))
        wc = const.tile([128, 4, 576], bf16)
        for kch in range(4):
            nc.sync.dma_start(wc[:, kch, :], wconv[kch])
        for pt in range(NPT):
            xt = sb.tile([128, 4, PT], bf16, tag="xt")
            for kch in range(4):
                nc.sync.dma_start(xt[:, kch, :], mk(xb, [[W4, 128], [1, PT]], kch*128*W4 + pt*PT))
            for mch in range(5):
                n = 128 if mch < 4 else 64
                cps = ps.tile([128, PT], f32, tag="cps")
                for kch in range(4):
                    nc.tensor.matmul(cps[0:n, :], wc[:, kch, mch*128:mch*128+n],
                                     xt[:, kch, :], start=(kch == 0), stop=(kch == 3))
                ot = sb.tile([128, PT], bf16, tag="cot")
                nc.scalar.activation(ot[0:n], cps[0:n], IDENT, bias=bcol[0:n, mch:mch+1])
                if mch < 2:
                    nc.sync.dma_start(mk(q1d if mch == 0 else q2d,
                                         [[W4, 128], [1, PT]], pt*PT), ot[:])
                elif mch < 4:
                    nc.sync.dma_start(mk(vd, [[W4, 128], [1, PT]],
                                         (mch-2)*128*W4 + pt*PT), ot[:])
                else:
                    nc.sync.dma_start(mk(ksd, [[W4, 64], [1, PT]], pt*PT), ot[0:64])

        ctx.close()
    nc.compile()
    return nc


def build_program2():
    nc = bacc.Bacc("TRN2", target_bir_lowering=False, debug=False, num_devices=8)
    def din(name, shape, dt=bf16):
        return nc.dram_tensor(name, shape, dt, kind="ExternalInput").ap()
    yd    = din("yd", [4*HD*W4])
    wproj = din("wproj", [2, 128, 512])
    bproj = din("bproj", [512], f32)
    out   = nc.dram_tensor("out", [512, W4], bf16, kind="ExternalOutput").ap()
    IDENT = mybir.ActivationFunctionType.Identity
    import contextlib
    ctx = contextlib.ExitStack()
    with tile.TileContext(nc) as tc, ctx:
        const = ctx.enter_context(tc.tile_pool(name="const", bufs=1))
        sb  = ctx.enter_context(tc.tile_pool(name="sb", bufs=3))
        ps  = ctx.enter_context(tc.tile_pool(name="ps", bufs=4, space="PSUM"))
        wp = const.tile([128, 2, 512], bf16)
        for kch in range(2):
            nc.sync.dma_start(wp[:, kch, :], wproj[kch])
        pcol = const.tile([128, 4], f32)
        for mch in range(4):
            nc.sync.dma_start(pcol[:, mch:mch+1], mk(bproj, [[1, 128], [1, 1]], mch*128))
        for pt in range(NPT):
            rhs = sb.tile([128, 2, PT], bf16, tag="prhs")
            for kch in range(2):
                nc.sync.dma_start(rhs[:, kch, :],
                                  mk(yd, [[W4, 128], [1, PT]], kch*128*W4 + pt*PT))
            for mch in range(4):
                pps = ps.tile([128, PT], f32, tag="pps")
                for kch in range(2):
                    nc.tensor.matmul(pps[:], wp[:, kch, mch*128:(mch+1)*128],
                                     rhs[:, kch, :], start=(kch == 0), stop=(kch == 1))
                po = sb.tile([128, PT], bf16, tag="po")
                nc.scalar.activation(po[:], pps[:], IDENT, bias=pcol[:, mch:mch+1])
                nc.sync.dma_start(mk(out, [[W4, 128], [1, PT]], mch*128*W4 + pt*PT), po[:])
        ctx.close()
    nc.compile()
    return nc


def host_prep(inputs, core):
    f = np.float32
    b = core // 2; hh = core % 2
    heads = list(range(hh*4, hh*4+4))
    def qch(h, s): return slice((h*2+s)*KD, (h*2+s)*KD+KD)
    def vch(h): return slice(h*HD, h*HD+HD)
    qk1_w, qk1_g, qk1_b = inputs['qk1_w'], inputs['qk1_g'], inputs['qk1_b']
    qk2_w, qk2_g, qk2_b = inputs['qk2_w'], inputs['qk2_g'], inputs['qk2_b']
    v_w, v_g, v_b = inputs['v_w'], inputs['v_g'], inputs['v_b']
    Wq1 = np.concatenate([qk1_w[qch(h,0)] * qk1_g[qch(h,0)][:,None] for h in heads])
    bq1 = np.concatenate([qk1_b[qch(h,0)] for h in heads])
    Wq2 = np.concatenate([qk2_w[qch(h,0)] * qk2_g[qch(h,0)][:,None] for h in heads])
    bq2 = np.concatenate([qk2_b[qch(h,0)] for h in heads])
    Wk1 = sum(qk1_w[qch(h,1)] * qk1_g[qch(h,1)][:,None] for h in range(NH))
    bk1 = sum(qk1_b[qch(h,1)] for h in range(NH))
    Wk2 = sum(qk2_w[qch(h,1)] * qk2_g[qch(h,1)][:,None] for h in range(NH))
    bk2 = sum(qk2_b[qch(h,1)] for h in range(NH))
    Wv = np.concatenate([v_w[vch(h)] * v_g[vch(h)][:,None] for h in heads])
    bv = np.concatenate([v_b[vch(h)] for h in heads])
    # conv weight tensor: (4 kch, 128 c, 576 out) = lhsT
    Wall = np.concatenate([Wq1, Wq2, Wv, Wk1, Wk2], axis=0).astype(f)  # (576, 512)
    wconv = Wall.T.reshape(4, 128, 576).astype(ml_dtypes.bfloat16)
    bconv = np.zeros(640, f)
    bconv[0:128] = bq1; bconv[128:256] = bq2; bconv[256:512] = bv
    bconv[512:544] = bk1; bconv[544:576] = bk2
    # pe weights (g folded), per chunk cols: (128, 6)
    wpe_full = np.concatenate([inputs['pe_w'][h*HD:(h+1)*HD] *
                               inputs['pe_g'][h*HD:(h+1)*HD][:,None] for h in heads])  # (256,3)
    wpet = np.zeros((128, 6), f)
    wpet[:, 0:3] = wpe_full[0:128]; wpet[:, 3:6] = wpe_full[128:256]
    # proj
    cols = np.concatenate([np.arange(h*HD, (h+1)*HD) for h in heads])
    Wp = (inputs['proj_w'][:, cols] * inputs['proj_g'][:, None]).astype(f)  # (512, 256)
    wproj = Wp.T.reshape(2, 128, 512).astype(ml_dtypes.bfloat16)
    if core % 2 == 0:
        bproj = (inputs['proj_b'] + inputs['proj_g'] *
                 (inputs['proj_w'] @ inputs['pe_b'])).astype(f)
    else:
        bproj = np.zeros(512, f)
    xbf = inputs['x'][b].reshape(4, 128, W4).astype(ml_dtypes.bfloat16)
    return {"xb": xbf, "wconv": wconv, "bconv": bconv, "wpe": wpet,
            "wpe_full": wpe_full, "wproj": wproj, "bproj": bproj}

_PROG = None
_PROG2 = None

_ATTN_JIT = None

def _attn_math(q1, q2, v, ks, xp):
    # q1,q2 (N,4,KD,12,12,12,12); v (N,4,HD,...); ks (N,2,KD,...); xp = jnp or np
    ks1 = ks[:, 0]; ks2 = ks[:, 1]
    def sm(z, ax):
        z = z - z.max(axis=ax, keepdims=True)
        e = xp.exp(z)
        return e / e.sum(axis=ax, keepdims=True)
    es = lambda s, *a: xp.einsum(s, *a, optimize=True) if xp is np else xp.einsum(s, *a)
    a1 = sm(es('chdijkl,cdIjkl->chIijkl', q1, ks1) * SCALE, 3)
    a2 = sm(es('chdijkl,cdiJkl->chJijkl', q2, ks2) * SCALE, 4)
    a3 = sm(es('chdijkl,cdijKl->chKijkl', q2, ks2) * SCALE, 5)
    a4 = sm(es('chdijkl,cdijkL->chLijkl', q2, ks2) * SCALE, 6)
    s1 = es('chdijkl,chIijkl->chdIjkl', v, a1)
    s2 = es('chdIjkl,chJIjkl->chdIJkl', s1, a2)
    m  = es('chKIJkl,chLIJKl->chLIJkl', a3, a4)
    return es('chdIJkl,chLIJkl->chdIJkL', s2, m)

def _attn_all(q1s, q2s, vs, kss, wpe):
    """Stacked bf16 over cores: q1s/q2s (8,128,W4), vs (8,256,W4), kss (8,64,W4),
    wpe (8,256,3) f32. Returns yd+pe as (8, 4*HD*W4) bf16. jax-CPU jit, numpy fallback."""
    global _ATTN_JIT
    G = (W0,)*4
    def full(q1r, q2r, vr, ksr, wper, xp):
        f32c = lambda t: t.astype(np.float32) if xp is np else t.astype('float32')
        q1 = f32c(q1r).reshape((8, 4, KD)+G); q2 = f32c(q2r).reshape((8, 4, KD)+G)
        v = f32c(vr); ks = f32c(ksr).reshape((8, 2, KD)+G)
        pe = v * wper[:, :, 1:2]
        if xp is np:
            pe[:, :, 1:] += v[:, :, :-1] * wper[:, :, 0:1]
            pe[:, :, :-1] += v[:, :, 1:] * wper[:, :, 2:3]
        else:
            pe = pe.at[:, :, 1:].add(v[:, :, :-1] * wper[:, :, 0:1])
            pe = pe.at[:, :, :-1].add(v[:, :, 1:] * wper[:, :, 2:3])
        y = _attn_math(q1, q2, v.reshape((8, 4, HD)+G), ks, xp)
        y = y.reshape(8, 4*HD*W4) + pe.reshape(8, 4*HD*W4)
        return y.astype(ml_dtypes.bfloat16) if xp is np else y.astype('bfloat16')
    try:
        import jax
        cpu = jax.devices("cpu")[0]
        if _ATTN_JIT is None:
            import jax.numpy as jnp
            _ATTN_JIT = jax.jit(lambda a, b, c, d, w: full(a, b, c, d, w, jnp))
        with jax.default_device(cpu):
            args = [jax.device_put(t, cpu) for t in (q1s, q2s, vs, kss, wpe)]
            return np.asarray(_ATTN_JIT(*args))
    except Exception:
        return full(q1s, q2s, vs, kss, wpe, np)

def kernel(**inputs):
    global _PROG, _PROG2
    inputs = {k: np.asarray(v) for k, v in inputs.items()}
    if _PROG is None:
        _PROG = build_program()
        _PROG2 = build_program2()
    preps = [host_prep(inputs, c) for c in range(8)]
    in1 = [{k: p[k] for k in ("xb", "wconv", "bconv")} for p in preps]
    r1 = bass_utils.run_bass_kernel_spmd(_PROG, in1, core_ids=list(range(8)))
    q1s = np.stack([r1.results[c]["q1d"].reshape(128, W4) for c in range(8)])
    q2s = np.stack([r1.results[c]["q2d"].reshape(128, W4) for c in range(8)])
    vs  = np.stack([r1.results[c]["vd"].reshape(256, W4) for c in range(8)])
    kss = np.stack([r1.results[c]["ksd"].reshape(64, W4) for c in range(8)])
    wpe = np.stack([preps[c]["wpe_full"] for c in range(8)]).astype(np.float32)
    yds = _attn_all(q1s, q2s, vs, kss, wpe)
    in2 = []
    for c in range(8):
        in2.append({"yd": yds[c],
                    "wproj": preps[c]["wproj"], "bproj": preps[c]["bproj"]})
    r2 = bass_utils.run_bass_kernel_spmd(_PROG2, in2, core_ids=list(range(8)))
    out = np.zeros((4, C, W4), np.float32)
    for c in range(8):
        out[c // 2] += r2.results[c]["out"].reshape(C, W4).astype(np.float32)
    return out



# revision 2
# speedup vs baseline: 7.4616x; 7.4616x over previous
# Trainium2 Bass kernel for nn_MultiHeadGridAttention1d (multi-head grid attention).
# 8 cores = (batch 0..4) x (head-half). Pipeline of device-resident stages:
#   x half-channels in (bf16, deduped)  -> all_gather(pair) on device
#   -> Bass conv program (q1,q2,v,ks)   -> jnp attention (neuronx-cc, on device)
#   -> Bass proj program (partial out)  -> psum_scatter(pair) -> bf16 out
# Only x (85MB) and the final output (85MB) cross the host<->device tunnel.
import os, sys
import numpy as np
import ml_dtypes

if '/opt/trn_rl_repo' not in sys.path:
    sys.path.insert(0, '/opt/trn_rl_repo')

import jax
import jax.numpy as jnp
from jax.sharding import Mesh, PartitionSpec, NamedSharding
from jax.experimental.shard_map import shard_map

import concourse.bass as bass
import concourse.tile as tile
from concourse import bacc, mybir
from concourse.bass2jax import _bass_exec_p, install_neuronx_cc_hook, partition_id_tensor

NH, KD, HD, C = 8, 32, 64, 512
W0 = 12; W4 = W0**4; G3 = W0**3; T = G3//4
SCALE = KD ** -0.5
PT = 432; NPT = W4 // PT
bf16 = mybir.dt.bfloat16; f32 = mybir.dt.float32
GROUPS = [[0, 1], [2, 3], [4, 5], [6, 7]]

def mk(ap, dims, off=0):
    return bass.AP(tensor=ap.tensor, offset=ap.offset + off, ap=dims)

def build_program():
    nc = bacc.Bacc("TRN2", target_bir_lowering=False, debug=False, num_devices=8)
    def din(name, shape, dt=bf16):
        return nc.dram_tensor(name, shape, dt, kind="ExternalInput").ap()
    xb    = din("xb", [4, 128, W4])
    wconv = din("wconv", [4, 128, 576])
    bconv = din("bconv", [640], f32)
    q1d = nc.dram_tensor("q1d", [128*W4], bf16, kind="ExternalOutput").ap()
    q2d = nc.dram_tensor("q2d", [128*W4], bf16, kind="ExternalOutput").ap()
    vd  = nc.dram_tensor("vd", [256*W4], bf16, kind="ExternalOutput").ap()
    ksd = nc.dram_tensor("ksd", [64*W4], bf16, kind="ExternalOutput").ap()

    IDENT = mybir.ActivationFunctionType.Identity
    import contextlib
    ctx = contextlib.ExitStack()
    with tile.TileContext(nc) as tc, ctx:
        const = ctx.enter_context(tc.tile_pool(name="const", bufs=1))
        sb  = ctx.enter_context(tc.tile_pool(name="sb", bufs=3))
        ps  = ctx.enter_context(tc.tile_pool(name="ps", bufs=2, space="PSUM"))
        bcol = const.tile([128, 5], f32)
        for mch in range(5):
            nc.sync.dma_start(bcol[:, mch:mch+1], mk(bconv, [[1, 128], [1, 1]], mch*128))
        wc = const.tile([128, 4, 576], bf16)
        for kch in range(4):
            nc.sync.dma_start(wc[:, kch, :], wconv[kch])
        for pt in range(NPT):
            xt = sb.tile([128, 4, PT], bf16, tag="xt")
            for kch in range(4):
                nc.sync.dma_start(xt[:, kch, :], mk(xb, [[W4, 128], [1, PT]], kch*128*W4 + pt*PT))
            for mch in range(5):
                n = 128 if mch < 4 else 64
                cps = ps.tile([128, PT], f32, tag="cps")
                for kch in range(4):
                    nc.tensor.matmul(cps[0:n, :], wc[:, kch, mch*128:mch*128+n],
                                     xt[:, kch, :], start=(kch == 0), stop=(kch == 3))
                ot = sb.tile([128, PT], bf16, tag="cot")
                nc.scalar.activation(ot[0:n], cps[0:n], IDENT, bias=bcol[0:n, mch:mch+1])
                if mch < 2:
                    nc.sync.dma_start(mk(q1d if mch == 0 else q2d,
                                         [[W4, 128], [1, PT]], pt*PT), ot[:])
                elif mch < 4:
                    nc.sync.dma_start(mk(vd, [[W4, 128], [1, PT]],
                                         (mch-2)*128*W4 + pt*PT), ot[:])
                else:
                    nc.sync.dma_start(mk(ksd, [[W4, 64], [1, PT]], pt*PT), ot[0:64])
        ctx.close()
    nc.compile()
    return nc


def build_program2():
    nc = bacc.Bacc("TRN2", target_bir_lowering=False, debug=False, num_devices=8)
    def din(name, shape, dt=bf16):
        return nc.dram_tensor(name, shape, dt, kind="ExternalInput").ap()
    yd    = din("yd", [4*HD*W4])
    wproj = din("wproj", [2, 128, 512])
    bproj = din("bproj", [512], f32)
    out   = nc.dram_tensor("out", [512, W4], bf16, kind="ExternalOutput").ap()
    IDENT = mybir.ActivationFunctionType.Identity
    import contextlib
    ctx = contextlib.ExitStack()
    with tile.TileContext(nc) as tc, ctx:
        const = ctx.enter_context(tc.tile_pool(name="const", bufs=1))
        sb  = ctx.enter_context(tc.tile_pool(name="sb", bufs=3))
        ps  = ctx.enter_context(tc.tile_pool(name="ps", bufs=4, space="PSUM"))
        wp = const.tile([128, 2, 512], bf16)
        for kch in range(2):
            nc.sync.dma_start(wp[:, kch, :], wproj[kch])
        pcol = const.tile([128, 4], f32)
        for mch in range(4):
            nc.sync.dma_start(pcol[:, mch:mch+1], mk(bproj, [[1, 128], [1, 1]], mch*128))
        for pt in range(NPT):
            rhs = sb.tile([128, 2, PT], bf16, tag="prhs")
            for kch in range(2):
                nc.sync.dma_start(rhs[:, kch, :],
                                  mk(yd, [[W4, 128], [1, PT]], kch*128*W4 + pt*PT))
            for mch in range(4):
                pps = ps.tile([128, PT], f32, tag="pps")
                for kch in range(2):
                    nc.tensor.matmul(pps[:], wp[:, kch, mch*128:(mch+1)*128],
                                     rhs[:, kch, :], start=(kch == 0), stop=(kch == 1))
                po = sb.tile([128, PT], bf16, tag="po")
                nc.scalar.activation(po[:], pps[:], IDENT, bias=pcol[:, mch:mch+1])
                nc.sync.dma_start(mk(out, [[W4, 128], [1, PT]], mch*128*W4 + pt*PT), po[:])
        ctx.close()
    nc.compile()
    return nc


def host_prep(inputs, core):
    f = np.float32
    hh = core % 2
    heads = list(range(hh*4, hh*4+4))
    def qch(h, s): return slice((h*2+s)*KD, (h*2+s)*KD+KD)
    def vch(h): return slice(h*HD, h*HD+HD)
    qk1_w, qk1_g, qk1_b = inputs['qk1_w'], inputs['qk1_g'], inputs['qk1_b']
    qk2_w, qk2_g, qk2_b = inputs['qk2_w'], inputs['qk2_g'], inputs['qk2_b']
    v_w, v_g, v_b = inputs['v_w'], inputs['v_g'], inputs['v_b']
    Wq1 = np.concatenate([qk1_w[qch(h,0)] * qk1_g[qch(h,0)][:,None] for h in heads])
    bq1 = np.concatenate([qk1_b[qch(h,0)] for h in heads])
    Wq2 = np.concatenate([qk2_w[qch(h,0)] * qk2_g[qch(h,0)][:,None] for h in heads])
    bq2 = np.concatenate([qk2_b[qch(h,0)] for h in heads])
    Wk1 = sum(qk1_w[qch(h,1)] * qk1_g[qch(h,1)][:,None] for h in range(NH))
    bk1 = sum(qk1_b[qch(h,1)] for h in range(NH))
    Wk2 = sum(qk2_w[qch(h,1)] * qk2_g[qch(h,1)][:,None] for h in range(NH))
    bk2 = sum(qk2_b[qch(h,1)] for h in range(NH))
    Wv = np.concatenate([v_w[vch(h)] * v_g[vch(h)][:,None] for h in heads])
    bv = np.concatenate([v_b[vch(h)] for h in heads])
    Wall = np.concatenate([Wq1, Wq2, Wv, Wk1, Wk2], axis=0).astype(f)  # (576, 512)
    wconv = Wall.T.reshape(4, 128, 576).astype(ml_dtypes.bfloat16)
    bconv = np.zeros(640, f)
    bconv[0:128] = bq1; bconv[128:256] = bq2; bconv[256:512] = bv
    bconv[512:544] = bk1; bconv[544:576] = bk2
    wpe_full = np.concatenate([inputs['pe_w'][h*HD:(h+1)*HD] *
                               inputs['pe_g'][h*HD:(h+1)*HD][:,None] for h in heads])  # (256,3)
    cols = np.concatenate([np.arange(h*HD, (h+1)*HD) for h in heads])
    Wp = (inputs['proj_w'][:, cols] * inputs['proj_g'][:, None]).astype(f)  # (512, 256)
    wproj = Wp.T.reshape(2, 128, 512).astype(ml_dtypes.bfloat16)
    if core % 2 == 0:
        bproj = (inputs['proj_b'] + inputs['proj_g'] *
                 (inputs['proj_w'] @ inputs['pe_b'])).astype(f)
    else:
        bproj = np.zeros(512, f)
    return {"wconv": wconv, "bconv": bconv, "wpe_full": wpe_full.astype(f),
            "wproj": wproj, "bproj": bproj}


# ---------------- device pipeline (cached jits) ----------------

def _bass_io_spec(nc):
    partition_name = nc.partition_id_tensor.name if nc.partition_id_tensor else None
    in_names, out_names, out_avals = [], [], []
    for alloc in nc.m.functions[0].allocations:
        if not isinstance(alloc, mybir.MemoryLocationSet):
            continue
        name = alloc.memorylocations[0].name
        if alloc.kind == "ExternalInput":
            if name != partition_name:
                in_names.append(name)
        elif alloc.kind == "ExternalOutput":
            out_names.append(name)
            out_avals.append(jax.core.ShapedArray(
                tuple(alloc.tensor_shape), mybir.dt.np(alloc.dtype)))
    return partition_name, in_names, out_names, out_avals


def _make_bass_jit(nc, mesh):
    """Sharded jit whose module is exactly params + bass_exec custom call."""
    partition_name, in_names, out_names, out_avals = _bass_io_spec(nc)
    n_params = len(in_names)
    all_in_names = list(in_names) + list(out_names)
    if partition_name is not None:
        all_in_names.append(partition_name)
    donate = tuple(range(n_params, n_params + len(out_names)))

    def _body(*args):
        operands = list(args)
        if partition_name is not None:
            operands.append(partition_id_tensor())
        outs = _bass_exec_p.bind(
            *operands, out_avals=tuple(out_avals),
            in_names=tuple(all_in_names), out_names=tuple(out_names),
            lowering_input_output_aliases=(), sim_require_finite=True,
            sim_require_nnan=True, nc=nc)
        return tuple(outs)

    nin = n_params + len(out_names)
    fn = jax.jit(
        shard_map(_body, mesh=mesh, in_specs=(PartitionSpec("core"),) * nin,
                  out_specs=(PartitionSpec("core"),) * len(out_names),
                  check_rep=False),
        donate_argnums=donate, keep_unused=True)
    return fn, out_avals


_ST = None

def _get_stages():
    global _ST
    if _ST is not None:
        return _ST
    install_neuronx_cc_hook()
    devices = jax.devices()[:8]
    mesh = Mesh(np.asarray(devices), ("core",))
    shard = NamedSharding(mesh, PartitionSpec("core"))

    conv_prog = build_program()
    proj_prog = build_program2()
    conv_jit, conv_avals = _make_bass_jit(conv_prog, mesh)
    proj_jit, proj_avals = _make_bass_jit(proj_prog, mesh)

    # on-device zero output buffers (donated into the bass jits each call)
    def _zeros(avals):
        return tuple(jnp.zeros((8 * a.shape[0],) + tuple(a.shape[1:]), a.dtype)
                     for a in avals)
    conv_zeros_jit = jax.jit(lambda: _zeros(conv_avals),
                             out_shardings=(shard,) * len(conv_avals))
    proj_zeros_jit = jax.jit(lambda: _zeros(proj_avals),
                             out_shardings=(shard,) * len(proj_avals))

    # x half-channels -> full x[b] per core
    def _gather_body(xh):  # (2, 128, W4) bf16
        xf = jax.lax.all_gather(xh, "core", axis_index_groups=GROUPS,
                                axis=0, tiled=True)  # (4, 128, W4)
        return xf
    gather_jit = jax.jit(shard_map(_gather_body, mesh=mesh,
                                   in_specs=(PartitionSpec("core"),),
                                   out_specs=PartitionSpec("core")))

    # attention on device (stock neuronx-cc jnp)
    G = (W0, W0, W0, W0)
    def _attn_body(q1r, q2r, vr, ksr, wper):
        q1 = q1r.astype(jnp.float32).reshape((4, KD) + G)
        q2 = q2r.astype(jnp.float32).reshape((4, KD) + G)
        v = vr.astype(jnp.float32).reshape(256, W4)
        ks = ksr.astype(jnp.float32).reshape((2, KD) + G)
        ks1, ks2 = ks[0], ks[1]
        sm = jax.nn.softmax
        a1 = sm(jnp.einsum('hdijkl,dIjkl->hIijkl', q1, ks1) * SCALE, axis=2)
        a2 = sm(jnp.einsum('hdijkl,diJkl->hJijkl', q2, ks2) * SCALE, axis=3)
        a3 = sm(jnp.einsum('hdijkl,dijKl->hKijkl', q2, ks2) * SCALE, axis=4)
        a4 = sm(jnp.einsum('hdijkl,dijkL->hLijkl', q2, ks2) * SCALE, axis=5)
        vh = v.reshape((4, HD) + G)
        s1 = jnp.einsum('hdijkl,hIijkl->hdIjkl', vh, a1)
        s2 = jnp.einsum('hdIjkl,hJIjkl->hdIJkl', s1, a2)
        m = jnp.einsum('hKIJkl,hLIJKl->hLIJkl', a3, a4)
        y = jnp.einsum('hdIJkl,hLIJkl->hdIJkL', s2, m)
        z = jnp.zeros((256, 1), jnp.float32)
        pe = (v * wper[:, 1:2]
              + jnp.concatenate([z, v[:, :-1] * wper[:, 0:1]], axis=1)
              + jnp.concatenate([v[:, 1:] * wper[:, 2:3], z], axis=1))
        yd = (y.reshape(256, W4) + pe).astype(jnp.bfloat16)
        return yd.reshape(4 * HD * W4)
    attn_jit = jax.jit(shard_map(_attn_body, mesh=mesh,
                                 in_specs=(PartitionSpec("core"),) * 5,
                                 out_specs=PartitionSpec("core")))

    # pair-sum of proj partials; each core keeps its half of the rows
    def _psum_body(o):  # (512, W4) bf16
        of = o.astype(jnp.float32)
        red = jax.lax.psum_scatter(of, "core", scatter_dimension=0,
                                   axis_index_groups=GROUPS, tiled=True)
        return red.astype(jnp.bfloat16)  # (256, W4)
    psum_jit = jax.jit(shard_map(_psum_body, mesh=mesh,
                                 in_specs=(PartitionSpec("core"),),
                                 out_specs=PartitionSpec("core")))

    _ST = dict(mesh=mesh, shard=shard, conv_jit=conv_jit, proj_jit=proj_jit,
               conv_zeros_jit=conv_zeros_jit, proj_zeros_jit=proj_zeros_jit,
               gather_jit=gather_jit, attn_jit=attn_jit, psum_jit=psum_jit)
    return _ST


def kernel(**inputs):
    inputs = {k: np.asarray(v) for k, v in inputs.items()}
    st = _get_stages()
    shard = st["shard"]

    preps = [host_prep(inputs, c) for c in range(8)]
    # x: each pair-core gets half the channels of its batch (deduped h2d)
    xbf = inputs['x'].astype(ml_dtypes.bfloat16).reshape(4, 4, 128, W4)
    xhalf = np.ascontiguousarray(xbf.reshape(4, 2, 2, 128, W4)
                                 .reshape(8, 2, 128, W4)).reshape(16, 128, W4)
    xg = jax.device_put(xhalf, shard)

    wconv_g = jax.device_put(np.concatenate([p["wconv"] for p in preps]), shard)
    bconv_g = jax.device_put(np.concatenate([p["bconv"] for p in preps]), shard)
    wpe_g = jax.device_put(np.concatenate([p["wpe_full"] for p in preps]), shard)
    wproj_g = jax.device_put(np.concatenate([p["wproj"] for p in preps]), shard)
    bproj_g = jax.device_put(np.concatenate([p["bproj"] for p in preps]), shard)

    xfull = st["gather_jit"](xg)
    cz = st["conv_zeros_jit"]()
    q1d, q2d, vd, ksd = st["conv_jit"](xfull, wconv_g, bconv_g, *cz)
    yd = st["attn_jit"](q1d, q2d, vd, ksd, wpe_g)
    pz = st["proj_zeros_jit"]()
    (outp,) = st["proj_jit"](yd, wproj_g, bproj_g, *pz)
    outs = st["psum_jit"](outp)  # global (2048, W4) bf16
    out = np.asarray(outs).astype(np.float32).reshape(4, C, W4)
    return out
